# revision 18
# baseline (speedup 1.0000x reference)
"""Trainium2 Bass kernel for nn_CNN_align: 4-conv siamese feature net +
correlation + TPS inlier-mask scoring, data-parallel over 8 NeuronCores.

kernel(**inputs) takes the FULL inputs (as produced by setup_inputs) and
returns the FULL [32] output. Internally: batch dim sharded 4 samples per
core, all weights replicated; everything (convs, correlation, regression,
12x12 TPS solve via batched Gauss-Jordan with one refinement step, mask
build, masked reduction) runs on-device in fp32.
"""
import os
import numpy as np

import concourse.bacc as bacc
import concourse.mybir as mybir
import concourse.tile as tile
from concourse.bass_utils import run_bass_kernel_spmd

dt = mybir.dt
AF = mybir.ActivationFunctionType
OP = mybir.AluOpType

N_CORES = 8
SPC = 4           # samples per core
IPC = SPC * 2     # images per core (A and B)
DEBUG = bool(int(os.environ.get("CK_DEBUG", "0")))

SRC_POINTS = np.array(
    [[0.0, 0.0], [0.5, 0.0], [1.0, 0.0], [0.0, 0.5], [0.5, 0.5], [1.0, 0.5],
     [0.0, 1.0], [5.0, 1.0], [1.0, 1.0]], dtype=np.float32)


# ----------------------------------------------------------------- host prep
def _prep_consts(W1, W2, W3, W4, Wr1, Wr2, Wd, bd):
    c = {}
    c["w1c"] = np.ascontiguousarray(W1.reshape(27, 64))
    w2p = np.zeros((3, 128, 128), np.float32)
    w2p[:, 0:64] = W2[0]
    w2p[:, 64:128] = W2[2]
    c["w2p"] = w2p
    c["w2m"] = np.ascontiguousarray(W2[1])                    # [3, 64, 128]
    c["w3t"] = np.stack(
        [np.stack([W3.reshape(9, 128, 256)[t, :, m * 128:(m + 1) * 128]
                   for m in range(2)]) for t in range(9)])
    c["w4t"] = np.stack([np.stack([np.stack(
        [W4.reshape(9, 256, 512)[t, kc * 128:(kc + 1) * 128, m * 128:(m + 1) * 128]
         for m in range(4)]) for kc in range(2)]) for t in range(9)])
    c["wr1t"] = np.stack([np.stack(
        [Wr1.reshape(49, 256, 128)[t, kc * 128:(kc + 1) * 128, :]
         for kc in range(2)]) for t in range(49)])
    c["wr2t"] = np.ascontiguousarray(Wr2.reshape(25, 128, 64))
    c["wdt"] = np.ascontiguousarray(Wd.reshape(36, 64, 18))
    c["bd"] = np.ascontiguousarray(bd.reshape(18, 1))
    c["ones128"] = np.ones((128, 1), np.float32)
    c["ident"] = np.eye(128, dtype=np.float32)
    c["e1"] = np.repeat(np.eye(16, dtype=np.float32), 16, axis=1)
    c["e2"] = np.tile(np.eye(16, dtype=np.float32), (1, 16))
    c["iota16"] = np.arange(16, dtype=np.float32).reshape(16, 1)
    xs = np.linspace(0, 1, 16).astype(np.float32)
    kl = np.arange(256)
    gx = xs[kl % 16]
    gy = xs[kl // 16]
    c["cgrid"] = np.stack([np.ones(256, np.float32), gx, gy])   # [3,256]
    c["g15"] = np.stack([gx, gy]) * np.float32(15.0)            # [2,256]
    c["srcx4"] = np.tile(SRC_POINTS[:, 0], (4, 1))              # [4,9]
    c["srcy4"] = np.tile(SRC_POINTS[:, 1], (4, 1))
    c["srcx36"] = np.tile(SRC_POINTS[:, 0], 4).reshape(36, 1)
    c["srcy36"] = np.tile(SRC_POINTS[:, 1], 4).reshape(36, 1)
    return c


def _prep_images(image_A, image_B):
    # -> [B*2, 4, 3, 129, 129] parity grids of end-padded images
    B = image_A.shape[0]
    imgs = np.stack([image_A, image_B], axis=1).reshape(2 * B, 256, 256, 3)
    P = np.zeros((2 * B, 257, 257, 3), np.float32)
    P[:, :256, :256, :] = imgs
    G = np.zeros((2 * B, 4, 3, 129, 129), np.float32)
    for py in (0, 1):
        for px in (0, 1):
            g = P[:, py::2, px::2, :]
            G[:, py * 2 + px, :, :g.shape[1], :g.shape[2]] = \
                g.transpose(0, 3, 1, 2)
    return G


# ------------------------------------------------------------- device build
def build(nc):
    f32 = dt.float32
    io = {}

    imgs = nc.dram_tensor("imgs", [IPC, 4, 3, 129, 129], f32,
                          kind="ExternalInput").ap()
    din = {}
    for nm, shp in [("w1c", [27, 64]), ("w2p", [3, 128, 128]),
                    ("w2m", [3, 64, 128]), ("w3t", [9, 2, 128, 128]),
                    ("w4t", [9, 2, 4, 128, 128]), ("wr1t", [49, 2, 128, 128]),
                    ("wr2t", [25, 128, 64]), ("wdt", [36, 64, 18]),
                    ("bd", [18, 1]), ("ones128", [128, 1]),
                    ("ident", [128, 128]), ("e1", [16, 256]),
                    ("e2", [16, 256]), ("iota16", [16, 1]),
                    ("cgrid", [3, 256]), ("g15", [2, 256]),
                    ("srcx4", [4, 9]), ("srcy4", [4, 9]),
                    ("srcx36", [36, 1]), ("srcy36", [36, 1])]:
        din[nm] = nc.dram_tensor(nm, shp, f32, kind="ExternalInput").ap()
    out = nc.dram_tensor("out", [4], f32, kind="ExternalOutput").ap()
    dscr = nc.dram_tensor("dscr", [2, 4, 9], f32).ap()
    wscr = nc.dram_tensor("wscr", [4, 12, 2], f32).ap()
    w0scr = nc.dram_tensor("w0scr", [4, 2], f32).ap()
    uscr = nc.dram_tensor("uscr", [36, 256], f32).ap()
    dbg = {}
    if DEBUG:
        for nm, shp in [("dbg_geo", [18, 4]), ("dbg_corr", [128, 4, 2, 256]),
                        ("dbg_feats", [128, 4, 8, 256]),
                        ("dbg_axy", [8, 256]), ("dbg_xfin", [4, 24]),
                        ("dbg_res", [128, 8]), ("dbg_u36", [36, 256]),
                        ("dbg_dst36", [36, 2]), ("dbg_uhat0", [12, 256]),
                        ("dbg_wt0", [12, 2])]:
            dbg[nm] = nc.dram_tensor(nm, shp, f32, kind="ExternalOutput").ap()

    with tile.TileContext(nc) as tc:
        with tc.tile_pool(name="glob", bufs=1) as gp, \
             tc.tile_pool(name="psum", bufs=1, space="PSUM") as pp:

            # ---- persistent tiles
            feats = gp.tile([128, 4, IPC, 256], f32)   # [ch, chunk, img, pos]
            ones_sb = gp.tile([128, 1], f32)
            ident_sb = gp.tile([128, 128], f32)
            e1_sb = gp.tile([16, 256], f32)
            e2_sb = gp.tile([16, 256], f32)
            iota_sb = gp.tile([16, 1], f32)
            cgrid_sb = gp.tile([3, 256], f32)
            gxrow = gp.tile([1, 256], f32)
            gyrow = gp.tile([1, 256], f32)
            g15x = gp.tile([1, 256], f32)
            g15y = gp.tile([1, 256], f32)
            g15_sb = gp.tile([2, 256], f32)
            srcx4_sb = gp.tile([4, 9], f32)
            srcy4_sb = gp.tile([4, 9], f32)
            srcx36_sb = gp.tile([36, 1], f32)
            srcy36_sb = gp.tile([36, 1], f32)
            bd_sb = gp.tile([18, 1], f32)
            for nm, t in [("ones128", ones_sb), ("ident", ident_sb),
                          ("e1", e1_sb), ("e2", e2_sb), ("iota16", iota_sb),
                          ("cgrid", cgrid_sb), ("g15", g15_sb),
                          ("srcx4", srcx4_sb), ("srcy4", srcy4_sb),
                          ("srcx36", srcx36_sb), ("srcy36", srcy36_sb),
                          ("bd", bd_sb)]:
                nc.sync.dma_start(t[:], din[nm])
            nc.sync.dma_start(gxrow[:], din["cgrid"][1:2, :])
            nc.sync.dma_start(gyrow[:], din["cgrid"][2:3, :])
            nc.sync.dma_start(g15x[:], din["g15"][0:1, :])
            nc.sync.dma_start(g15y[:], din["g15"][1:2, :])

            # ================= STAGE A: conv pipeline =================
            with tc.tile_pool(name="conv", bufs=1) as cp:
                w1t = cp.tile([128, 64], f32)
                for g in range(4):
                    nc.sync.dma_start(w1t[32 * g:32 * g + 27, :], din["w1c"])
                w2p_sb = cp.tile([128, 3, 128], f32)
                nc.sync.dma_start(w2p_sb[:], din["w2p"].transpose([1, 0, 2]))
                w2m_sb = cp.tile([64, 3, 128], f32)
                nc.sync.dma_start(w2m_sb[:], din["w2m"].transpose([1, 0, 2]))
                w3_sb = cp.tile([128, 18, 128], f32)
                nc.sync.dma_start(
                    w3_sb[:], din["w3t"].transpose([2, 0, 1, 3])
                    .rearrange("p a b c -> p (a b) c"))
                w4_sb = cp.tile([128, 72, 128], f32)
                nc.sync.dma_start(
                    w4_sb[:], din["w4t"].transpose([3, 0, 1, 2, 4])
                    .rearrange("p a b c d -> p (a b c) d"))

                for i in range(IPC):
                    ic = cp.tile([128, 4096], f32, tag="ic")
                    icr = ic[:].rearrange("(g q) n -> g q n", g=4)
                    for ky in range(3):
                        for kx in range(3):
                            t = ky * 3 + kx
                            gidx = (ky & 1) * 2 + (kx & 1)
                            r0 = 1 if ky == 2 else 0
                            c0 = 1 if kx == 2 else 0
                            for ch in range(3):
                                src = imgs[i, gidx, ch,
                                           r0:r0 + 128, c0:c0 + 128] \
                                    .rearrange("(g yy) x -> g yy x", g=4)
                                dst = icr[:, 3 * t + ch, :] \
                                    .rearrange("g (yy x) -> g yy x", yy=32)
                                nc.sync.dma_start(dst, src)

                    c1 = cp.tile([128, 129, 129], f32, tag="c1")
                    nc.vector.memset(c1[0:64, 128, :], 0.0)
                    nc.vector.memset(c1[0:64, :, 128], 0.0)
                    for n in range(8):
                        for g in range(4):
                            ps = pp.tile([64, 512], f32, tag=f"c1g{g}",
                                         bufs=1)
                            nc.tensor.matmul(
                                ps[:], w1t[32 * g:32 * g + 27, :],
                                ic[32 * g:32 * g + 27, 512 * n:512 * (n + 1)],
                                start=True, stop=True,
                                tile_position=(32 * g, 0))
                            y0 = 32 * g + 4 * n
                            dst = c1[0:64, y0:y0 + 4, 0:128]
                            if (n + g) % 2 == 0:
                                nc.scalar.activation(dst, ps[:], AF.Relu)
                            else:
                                nc.vector.tensor_scalar_max(dst, ps[:], 0.0)
                    # duplicate shifted by 2 rows into upper partitions
                    # (chunked so conv2 can start before the whole copy lands)
                    for g2 in range(4):
                        r0, r1 = 32 * g2, min(32 * g2 + 32, 127)
                        nc.sync.dma_start(c1[64:128, r0:r1, :],
                                          c1[0:64, r0 + 2:r1 + 2, :])

                    # ---- conv2
                    c2 = cp.tile([128, 65, 65], f32, tag="c2")
                    nc.vector.memset(c2[:, 64, :], 0.0)
                    nc.vector.memset(c2[:, :, 64], 0.0)
                    for n in range(8):
                        ps = pp.tile([128, 512], f32, tag="big", bufs=3)
                        y0 = 8 * n
                        for k in range(3):
                            rhs = c1[:, 2 * y0:2 * y0 + 16:2, k:k + 127:2]
                            nc.tensor.matmul(ps[:], w2p_sb[:, k, :], rhs,
                                             start=(k == 0), stop=False)
                        for k in range(3):
                            rhs = c1[0:64, 2 * y0 + 1:2 * y0 + 17:2,
                                     k:k + 127:2]
                            nc.tensor.matmul(ps[:], w2m_sb[:, k, :], rhs,
                                             start=False, stop=(k == 2))
                        dst = c2[:, y0:y0 + 8, 0:64]
                        if n % 2 == 0:
                            nc.scalar.activation(dst, ps[:], AF.Relu)
                        else:
                            nc.vector.tensor_scalar_max(dst, ps[:], 0.0)

                    # ---- conv3
                    c3a = cp.tile([128, 33, 33], f32, tag="c3a")
                    c3b = cp.tile([128, 33, 33], f32, tag="c3b")
                    for cten, m in ((c3a, 0), (c3b, 1)):
                        nc.vector.memset(cten[:, 32, :], 0.0)
                        nc.vector.memset(cten[:, :, 32], 0.0)
                        for n in range(2):
                            ps = pp.tile([128, 512], f32, tag="big")
                            y0 = 16 * n
                            first = True
                            for ky in range(3):
                                for kx in range(3):
                                    t = ky * 3 + kx
                                    rhs = c2[:, 2 * y0 + ky:2 * y0 + ky + 31:2,
                                             kx:kx + 63:2]
                                    nc.tensor.matmul(
                                        ps[:], w3_sb[:, t * 2 + m, :], rhs,
                                        start=first, stop=(t == 8))
                                    first = False
                            dst = cten[:, y0:y0 + 16, 0:32]
                            if n % 2 == 0:
                                nc.scalar.activation(dst, ps[:], AF.Relu)
                            else:
                                nc.vector.tensor_scalar_max(dst, ps[:], 0.0)

                    # ---- conv4 (+relu into feats)
                    for m in range(4):
                        ps = pp.tile([128, 256], f32, tag="big")
                        first = True
                        for ky in range(3):
                            for kx in range(3):
                                t = ky * 3 + kx
                                for kc, cten in ((0, c3a), (1, c3b)):
                                    rhs = cten[:, ky:ky + 31:2, kx:kx + 31:2]
                                    nc.tensor.matmul(
                                        ps[:],
                                        w4_sb[:, (t * 2 + kc) * 4 + m, :],
                                        rhs, start=first,
                                        stop=(t == 8 and kc == 1))
                                    first = False
                        dst = feats[:, m, i, :]
                        if m % 2 == 0:
                            nc.scalar.activation(dst, ps[:], AF.Relu)
                        else:
                            nc.vector.tensor_scalar_max(dst, ps[:], 0.0)

                    # ---- channelwise l2 normalization
                    sq = cp.tile([128, 256], f32, tag="sq")
                    psn = pp.tile([1, 256], f32, tag="sm")
                    for m in range(4):
                        nc.scalar.activation(sq[:], feats[:, m, i, :],
                                             AF.Square)
                        nc.tensor.matmul(psn[:], ones_sb[:], sq[:],
                                         start=(m == 0), stop=(m == 3))
                    nrm = cp.tile([1, 256], f32, tag="nrm")
                    nc.scalar.activation(nrm[:], psn[:], AF.Sqrt)
                    t1 = cp.tile([1, 256], f32, tag="nt1")
                    nc.vector.tensor_scalar_max(t1[:], nrm[:], 1e-30)
                    rec = cp.tile([1, 256], f32, tag="nt2")
                    nc.vector.reciprocal(rec[:], t1[:])
                    nc.vector.tensor_tensor(t1[:], psn[:], rec[:], OP.mult)
                    nc.vector.tensor_tensor(t1[:], t1[:], nrm[:], OP.add)
                    # r1 = 0.5*(r + n2/r); inv = 1/(r1 + 1e-6)
                    nc.vector.tensor_scalar(t1[:], t1[:], 0.5, 1e-6,
                                            OP.mult, OP.add)
                    nc.vector.reciprocal(rec[:], t1[:])
                    invb = cp.tile([128, 256], f32, tag="invb")
                    nc.gpsimd.partition_broadcast(invb[:], rec[:])
                    for m in range(4):
                        nc.vector.tensor_tensor(feats[:, m, i, :],
                                                feats[:, m, i, :], invb[:],
                                                OP.mult)

            # ================= STAGE B: corr + regression =================
            with tc.tile_pool(name="post", bufs=1) as qp:
                wr1_sb = qp.tile([128, 98, 128], f32)
                nc.sync.dma_start(
                    wr1_sb[:], din["wr1t"].transpose([2, 0, 1, 3])
                    .rearrange("p a b c -> p (a b) c"))
                wr2_sb = qp.tile([128, 25, 64], f32)
                nc.sync.dma_start(wr2_sb[:], din["wr2t"].transpose([1, 0, 2]))
                wd_sb = qp.tile([64, 36, 18], f32)
                nc.sync.dma_start(wd_sb[:], din["wdt"].transpose([1, 0, 2]))

                corr = qp.tile([128, 4, 2, 256], f32)
                rt = qp.tile([128, 4, 2, 256], f32)
                for s in range(SPC):
                    for m in range(2):
                        ps = pp.tile([128, 256], f32, tag="big")
                        for ch in range(4):
                            nc.tensor.matmul(
                                ps[:],
                                feats[:, ch, 2 * s, 128 * m:128 * (m + 1)],
                                feats[:, ch, 2 * s + 1, :],
                                start=(ch == 0), stop=(ch == 3))
                        nc.vector.tensor_copy(corr[:, s, m, :], ps[:])
                    for m in range(2):
                        for kc in range(2):
                            pst = pp.tile([128, 128], f32, tag="big")
                            nc.tensor.transpose(
                                pst[:], corr[:, s, m, 128 * kc:128 * (kc + 1)],
                                ident_sb[:])
                            nc.scalar.activation(
                                rt[:, s, kc, 128 * m:128 * (m + 1)], pst[:],
                                AF.Copy)
                if DEBUG:
                    nc.sync.dma_start(dbg["dbg_corr"], corr[:])
                    nc.sync.dma_start(dbg["dbg_feats"], feats[:])

                rtv = rt[:].rearrange("p s c (a b) -> p s c a b", a=16)
                psw = pp.tile([128, 400], f32, tag="big")
                first = True
                for ky in range(7):
                    for kx in range(7):
                        t = ky * 7 + kx
                        for kc in range(2):
                            rhs = rtv[:, :, kc, ky:ky + 10, kx:kx + 10]
                            nc.tensor.matmul(psw[:], wr1_sb[:, t * 2 + kc, :],
                                             rhs, start=first,
                                             stop=(t == 48 and kc == 1))
                            first = False
                r1t = qp.tile([128, 4, 10, 10], f32)
                nc.scalar.activation(r1t[:], psw[:], AF.Relu)

                psw2 = pp.tile([64, 144], f32, tag="big")
                first = True
                for ky in range(5):
                    for kx in range(5):
                        t = ky * 5 + kx
                        rhs = r1t[:, :, ky:ky + 6, kx:kx + 6]
                        nc.tensor.matmul(psw2[:], wr2_sb[:, t, :], rhs,
                                         start=first, stop=(t == 24))
                        first = False
                r2t = qp.tile([64, 4, 36], f32)
                nc.scalar.activation(r2t[:], psw2[:], AF.Relu)

                psd = pp.tile([18, 4], f32, tag="sm")
                for ij in range(36):
                    nc.tensor.matmul(psd[:], wd_sb[:, ij, :], r2t[:, :, ij],
                                     start=(ij == 0), stop=(ij == 35))
                geo32 = qp.tile([32, 32], f32)
                nc.vector.memset(geo32[:], 0.0)
                nc.vector.tensor_scalar(geo32[0:18, 0:4], psd[:],
                                        bd_sb[:], None, OP.add)
                if DEBUG:
                    nc.sync.dma_start(dbg["dbg_geo"], geo32[0:18, 0:4])

                # ================= STAGE C: TPS solve + masks ===========
                geot = qp.tile([32, 32], f32)
                nc.vector.transpose(geot[:], geo32[:])
                gT = geot[0:4, 0:18]

                dstx = qp.tile([4, 9], f32)
                dsty = qp.tile([4, 9], f32)
                nc.vector.tensor_tensor(dstx[:], geot[0:4, 0:18:2],
                                        srcx4_sb[:], OP.add)
                nc.vector.tensor_tensor(dsty[:], geot[0:4, 1:18:2],
                                        srcy4_sb[:], OP.add)

                tab = qp.tile([4, 12, 14], f32)
                tab0 = qp.tile([4, 12, 14], f32)
                nc.vector.memset(tab[:], 0.0)
                nc.vector.memset(tab[:, 0, 0:9], 1.0)
                nc.vector.tensor_copy(tab[:, 1, 0:9], dstx[:])
                nc.vector.tensor_copy(tab[:, 2, 0:9], dsty[:])
                nc.vector.memset(tab[:, 3:12, 9], 1.0)
                nc.vector.tensor_copy(tab[:, 3:12, 10], dstx[:])
                nc.vector.tensor_copy(tab[:, 3:12, 11], dsty[:])
                nc.vector.tensor_scalar_mul(tab[:, 3:12, 12],
                                            geot[0:4, 0:18:2], -1.0)
                nc.vector.tensor_scalar_mul(tab[:, 3:12, 13],
                                            geot[0:4, 1:18:2], -1.0)

                # K block: U(dist) into tab[:, 3:12, 0:9]
                def build_u(out_ap, dxa, dya, tmps, shape):
                    # out = (s2+1e-12) * 0.5 * ln(s2 + 1e-12 + 2e-6*r + 1e-12)
                    t_s2, t_r, t_q = tmps
                    nc.vector.tensor_tensor(t_s2, dxa, dxa, OP.mult)
                    nc.vector.tensor_tensor(t_r, dya, dya, OP.mult)
                    nc.vector.tensor_tensor(t_s2, t_s2, t_r, OP.add)
                    nc.vector.tensor_scalar_add(t_s2, t_s2, 1e-12)
                    nc.scalar.activation(t_r, t_s2, AF.Sqrt)
                    nc.vector.tensor_scalar(t_q, t_r, 2e-6, 1e-12,
                                            OP.mult, OP.add)
                    nc.vector.tensor_tensor(t_q, t_q, t_s2, OP.add)
                    nc.scalar.activation(t_q, t_q, AF.Ln)
                    nc.vector.scalar_tensor_tensor(out_ap, t_s2, 0.5, t_q,
                                                   OP.mult, OP.mult)

                dx9 = qp.tile([4, 9, 9], f32)
                dy9 = qp.tile([4, 9, 9], f32)
                u_s2 = qp.tile([4, 9, 9], f32)
                u_r = qp.tile([4, 9, 9], f32)
                u_q = qp.tile([4, 9, 9], f32)
                nc.vector.tensor_tensor(
                    dx9[:], dstx[:].unsqueeze(2).to_broadcast((4, 9, 9)),
                    dstx[:].unsqueeze(1).to_broadcast((4, 9, 9)), OP.subtract)
                nc.vector.tensor_tensor(
                    dy9[:], dsty[:].unsqueeze(2).to_broadcast((4, 9, 9)),
                    dsty[:].unsqueeze(1).to_broadcast((4, 9, 9)), OP.subtract)
                build_u(tab[:, 3:12, 0:9], dx9[:], dy9[:],
                        (u_s2[:], u_r[:], u_q[:]), (4, 9, 9))

                nc.vector.tensor_copy(tab0[:], tab[:])

                f_t = qp.tile([4, 12], f32)
                upd = qp.tile([4, 12, 14], f32)
                rec1 = qp.tile([4, 1], f32)

                def gj(tb):
                    for k in range(12):
                        nc.vector.reciprocal(rec1[:], tb[:, k, k:k + 1])
                        nc.vector.tensor_scalar(tb[:, k, :], tb[:, k, :],
                                                rec1[:], None, OP.mult)
                        nc.vector.tensor_copy(f_t[:], tb[:, :, k])
                        nc.vector.memset(f_t[:, k:k + 1], 0.0)
                        nc.vector.tensor_tensor(
                            upd[:],
                            f_t[:].unsqueeze(2).to_broadcast((4, 12, 14)),
                            tb[:, k, :].unsqueeze(1).to_broadcast((4, 12, 14)),
                            OP.mult)
                        nc.vector.tensor_tensor(tb[:], tb[:], upd[:],
                                                OP.subtract)

                gj(tab[:])
                # refinement: r = v - A0 x ; dx = GJ(A0, r); x += dx
                t3 = qp.tile([4, 12, 2, 12], f32)
                nc.vector.tensor_tensor(
                    t3[:],
                    tab0[:, :, 0:12].unsqueeze(2).to_broadcast((4, 12, 2, 12)),
                    tab[:, :, 12:14].transpose([0, 2, 1]).unsqueeze(1)
                    .to_broadcast((4, 12, 2, 12)),
                    OP.mult)
                rax = qp.tile([4, 12, 2], f32)
                nc.vector.tensor_reduce(rax[:], t3[:], mybir.AxisListType.X,
                                        OP.add)
                nc.vector.tensor_tensor(tab0[:, :, 12:14], tab0[:, :, 12:14],
                                        rax[:], OP.subtract)
                gj(tab0[:])
                xfin = qp.tile([4, 12, 2], f32)
                nc.vector.tensor_tensor(xfin[:], tab[:, :, 12:14],
                                        tab0[:, :, 12:14], OP.add)
                if DEBUG:
                    nc.sync.dma_start(dbg["dbg_xfin"],
                                      xfin[:].rearrange("p a b -> p (a b)"))

                w0 = qp.tile([4, 2], f32)
                nc.vector.tensor_reduce(w0[:],
                                        xfin[:, 1:9, :].transpose([0, 2, 1]),
                                        mybir.AxisListType.X, OP.add)
                nc.vector.tensor_scalar_mul(w0[:], w0[:], -1.0)

                # U36 for grid points
                gx36 = qp.tile([36, 256], f32)
                gy36 = qp.tile([36, 256], f32)
                nc.gpsimd.partition_broadcast(gx36[:], gxrow[:])
                nc.gpsimd.partition_broadcast(gy36[:], gyrow[:])
                dst36x = qp.tile([36, 1], f32)
                dst36y = qp.tile([36, 1], f32)
                # gather dst into (s,c) partition layout via DRAM bounce
                nc.sync.dma_start(dscr[0], dstx[:])
                nc.sync.dma_start(dscr[1], dsty[:])
                nc.sync.dma_start(
                    dst36x[:], dscr[0].rearrange("s c -> (s c)").unsqueeze(1))
                nc.sync.dma_start(
                    dst36y[:], dscr[1].rearrange("s c -> (s c)").unsqueeze(1))
                u36 = qp.tile([36, 256], f32)
                w_s2 = qp.tile([36, 256], f32)
                w_r = qp.tile([36, 256], f32)
                w_q = qp.tile([36, 256], f32)
                nc.vector.tensor_scalar(gx36[:], gx36[:], dst36x[:], None,
                                        OP.subtract)
                nc.vector.tensor_scalar(gy36[:], gy36[:], dst36y[:], None,
                                        OP.subtract)
                build_u(u36[:], gx36[:], gy36[:],
                        (w_s2[:], w_r[:], w_q[:]), (36, 256))
                if DEBUG:
                    nc.sync.dma_start(dbg["dbg_u36"], u36[:])
                    nc.sync.dma_start(dbg["dbg_dst36"][:, 0:1], dst36x[:])
                    nc.sync.dma_start(dbg["dbg_dst36"][:, 1:2], dst36y[:])

                res_t = qp.tile([128, 8], f32)
                axy_all = qp.tile([8, 256], f32)
                for s in range(SPC):
                    uhat = qp.tile([12, 256], f32, tag="uhat")
                    nc.sync.dma_start(uhat[0:9, :], u36[9 * s:9 * s + 9, :])
                    nc.sync.dma_start(uhat[9:12, :], din["cgrid"])
                    wt = qp.tile([12, 2], f32, tag="wt")
                    nc.sync.dma_start(wt[1:12, :], xfin[s:s + 1, 1:12, :])
                    nc.sync.dma_start(wt[0:1, :], w0[s:s + 1, :])
                    if DEBUG and s == 0:
                        nc.sync.dma_start(dbg["dbg_uhat0"], uhat[:])
                        nc.sync.dma_start(dbg["dbg_wt0"], wt[:])
                    myx = qp.tile([16, 2, 256], f32, tag="myx")
                    for d in range(2):  # d=0: mx (from ax), d=1: my (ay)
                        psz = pp.tile([1, 256], f32, tag="sm")
                        nc.tensor.matmul(psz[:], wt[:, d:d + 1], uhat[:],
                                         start=True, stop=True)
                        arow = qp.tile([1, 256], f32, tag="arow")
                        nc.vector.scalar_tensor_tensor(
                            arow[:], psz[:], 15.0,
                            g15x[:] if d == 0 else g15y[:],
                            OP.mult, OP.add)
                        if DEBUG:
                            nc.sync.dma_start(
                                axy_all[2 * s + d:2 * s + d + 1, :], arow[:])
                        bcast = qp.tile([16, 256], f32, tag="bc")
                        nc.gpsimd.partition_broadcast(bcast[:], arow[:])
                        nc.vector.tensor_scalar(myx[:, d, :], bcast[:],
                                                iota_sb[:], None, OP.subtract)
                        nc.scalar.activation(myx[:, d, :], myx[:, d, :],
                                             AF.Abs)
                        nc.vector.tensor_scalar(myx[:, d, :], myx[:, d, :],
                                                1.0, None, OP.is_le)
                    for m in range(2):
                        psy = pp.tile([128, 256], f32, tag="big")
                        psx = pp.tile([128, 256], f32, tag="big")
                        nc.tensor.matmul(psy[:],
                                         e1_sb[:, 128 * m:128 * (m + 1)],
                                         myx[:, 1, :], start=True, stop=True)
                        nc.tensor.matmul(psx[:],
                                         e2_sb[:, 128 * m:128 * (m + 1)],
                                         myx[:, 0, :], start=True, stop=True)
                        prod = qp.tile([128, 256], f32, tag="prod")
                        nc.vector.tensor_tensor(prod[:], corr[:, s, m, :],
                                                psy[:], OP.mult)
                        nc.vector.tensor_tensor(prod[:], prod[:], psx[:],
                                                OP.mult)
                        nc.vector.tensor_reduce(res_t[:, 2 * s + m:2 * s + m + 1],
                                                prod[:], mybir.AxisListType.X,
                                                OP.add)
                if DEBUG:
                    nc.sync.dma_start(dbg["dbg_axy"], axy_all[:])
                    nc.sync.dma_start(dbg["dbg_res"], res_t[:])

                psf = pp.tile([1, 8], f32, tag="sm")
                nc.tensor.matmul(psf[:], ones_sb[:], res_t[:],
                                 start=True, stop=True)
                f8 = qp.tile([1, 8], f32)
                nc.vector.tensor_copy(f8[:], psf[:])
                o4 = qp.tile([1, 4], f32)
                nc.vector.tensor_tensor(o4[:], f8[:, 0:8:2], f8[:, 1:8:2],
                                        OP.add)
                nc.sync.dma_start(out, o4[0, :])

    nc.compile()
    return nc


# ----------------------------------------------------------------- kernel()
_BUILD_CACHE = {}


def _get_program():
    if "nc" not in _BUILD_CACHE:
        nc = bacc.Bacc("TRN2", target_bir_lowering=False, debug=False,
                       num_devices=N_CORES)
        build(nc)
        _BUILD_CACHE["nc"] = nc
    return _BUILD_CACHE["nc"]


def kernel(**inputs):
    consts = _prep_consts(inputs["W1"], inputs["W2"], inputs["W3"],
                          inputs["W4"], inputs["Wr1"], inputs["Wr2"],
                          inputs["Wd"], inputs["bd"])
    G = _prep_images(np.asarray(inputs["image_A"], dtype=np.float32),
                     np.asarray(inputs["image_B"], dtype=np.float32))
    nc = _get_program()
    in_maps = []
    for c in range(N_CORES):
        m = {"imgs": np.ascontiguousarray(G[c * IPC:(c + 1) * IPC])}
        for k, v in consts.items():
            m[k] = np.ascontiguousarray(v.astype(np.float32))
        in_maps.append(m)
    res = run_bass_kernel_spmd(nc, in_maps, list(range(N_CORES)))
    out = np.concatenate([res.results[c]["out"] for c in range(N_CORES)])
    return out.astype(np.float32)


if __name__ == "__main__":
    import sys
    sys.path.insert(0, os.path.dirname(os.path.abspath(__file__)))
